# revision 7
# baseline (speedup 1.0000x reference)
"""Trainium2 Bass kernel for nn_DiffKS (time-varying FIR / InvertLPC forward).

Computes x[b,t] = y[b,t] + sum_k A[b,t,k] * y_padded[b, t+N-1-k]
with B=4, T=64000, N=588, y_padded = concat(flip(zi), y).

Sharding: 8 cores = 4 batches x 2 halves of T (each core: 32000 time steps).
Per-core layout: SBUF partition p owns 250 contiguous time steps
(t = 250*p + I, I in [0,250)).  The shifted-signal operand for output sample
I is a contiguous 588-column slice of a tiny per-partition reversed window
tile v[128, 837] (v[p] = reversed y_padded[250p : 250p+837] of the shard),
so no shift/Hankel materialization is needed on-device.  One fused DVE
tensor_tensor_reduce per output sample does multiply + sum in a single pass,
while A streams from HBM in large contiguous DMAs (memory-bound roofline).
"""

import os
import sys

import numpy as np

for _p in ("/opt/trn_rl_repo", "/opt/pypackages"):
    if _p not in sys.path:
        sys.path.append(_p)

B, T, N = 4, 64000, 588
NCORES = 8
TS = T // 2          # 32000 time steps per core shard
P = 128              # partitions
NI = TS // P         # 250 samples per partition
VW = NI + N - 1      # 837 window columns per partition
F = int(os.environ.get("K_F", "5"))       # samples per A-chunk DMA tile
BUFS = int(os.environ.get("K_BUFS", "12"))  # A-tile pool depth
NCHUNK = NI // F

_cached = {}


def _build_program():
    import concourse.bass as bass
    import concourse.tile as tile
    from concourse import bacc, mybir

    f32 = mybir.dt.float32
    nc = bacc.Bacc("TRN2", target_bir_lowering=False, debug=False,
                   num_devices=NCORES)

    a_dram = nc.dram_tensor("a_s", [TS, N], f32, kind="ExternalInput").ap()
    v_dram = nc.dram_tensor("v", [P, VW], f32, kind="ExternalInput").ap()
    r_dram = nc.dram_tensor("r", [P, NI], f32, kind="ExternalOutput").ap()

    # [TS, N] -> [P, NI, N]: partition p holds rows 250p .. 250p+249
    a_r = a_dram.rearrange("(p n) k -> p n k", p=P)

    with tile.TileContext(nc) as tc:
        with (
            tc.tile_pool(name="a", bufs=BUFS) as a_pool,
            tc.tile_pool(name="prod", bufs=2) as p_pool,
            tc.tile_pool(name="const", bufs=1) as c_pool,
        ):
            v_tile = c_pool.tile([P, VW], f32)
            nc.sync.dma_start(v_tile[:], v_dram[:])
            out_tile = c_pool.tile([P, NI], f32)

            dual = os.environ.get("K_DUAL", "0") == "1"
            for f in range(NCHUNK):
                a_tile = a_pool.tile([P, F * N], f32)
                eng = nc.scalar if (dual and f % 2) else nc.sync
                eng.dma_start(a_tile[:], a_r[:, f * F:(f + 1) * F, :])
                for i in range(F):
                    I = f * F + i
                    prod = p_pool.tile([P, N], f32)
                    nc.vector.scalar_tensor_tensor(
                        out=prod[:],
                        in0=a_tile[:, i * N:(i + 1) * N],
                        scalar=1.0,
                        in1=v_tile[:, (NI - 1 - I):(NI - 1 - I) + N],
                        op0=mybir.AluOpType.mult,
                        op1=mybir.AluOpType.mult,
                        accum_out=out_tile[:, I:I + 1],
                    )
            nc.sync.dma_start(r_dram[:], out_tile[:])
    nc.compile()
    return nc


def _get_program():
    if "nc" not in _cached:
        _cached["nc"] = _build_program()
    return _cached["nc"]


def _make_in_maps(y, A, zi):
    from numpy.lib.stride_tricks import sliding_window_view

    y_pad = np.concatenate([zi[:, ::-1], y], axis=1)  # [B, N+T]
    in_maps = []
    for c in range(NCORES):
        b, h = divmod(c, 2)
        base = h * TS
        a_s = A[b, base:base + TS, :]
        seg = y_pad[b, base:base + (TS - NI) + VW]          # [32587]
        v = sliding_window_view(seg, VW)[::NI]               # [128, 837]
        v = np.ascontiguousarray(v[:, ::-1])                 # reversed windows
        in_maps.append({"a_s": np.ascontiguousarray(a_s), "v": v})
    return in_maps


def _run(nc, in_maps, trace=False, **kw):
    from concourse.bass_utils import run_bass_kernel_spmd

    return run_bass_kernel_spmd(nc, in_maps, list(range(NCORES)),
                                trace=trace, **kw)


def kernel(y, A, zi):
    y = np.asarray(y, dtype=np.float32)
    A = np.asarray(A, dtype=np.float32)
    zi = np.asarray(zi, dtype=np.float32)

    nc = _get_program()
    res = _run(nc, _make_in_maps(y, A, zi))

    x = np.empty((B, T), dtype=np.float32)
    for c in range(NCORES):
        b, h = divmod(c, 2)
        base = h * TS
        r = res.results[c]["r"].reshape(TS)   # [128,250] -> t = 250p + I
        x[b, base:base + TS] = y[b, base:base + TS] + r
    return x


# revision 8
# speedup vs baseline: 1.0104x; 1.0104x over previous
"""Trainium2 Bass kernel for nn_DiffKS (time-varying FIR / InvertLPC forward).

Computes x[b,t] = y[b,t] + sum_k A[b,t,k] * y_padded[b, t+N-1-k]
with B=4, T=64000, N=588, y_padded = concat(flip(zi), y).

Sharding: 8 cores = 4 batches x 2 halves of T (each core: 32000 time steps).
Per-core layout: SBUF partition p owns 250 contiguous time steps
(t = 250*p + I, I in [0,250)).  The shifted-signal operand for output sample
I is a contiguous 588-column slice of a tiny per-partition reversed window
tile v[128, 837] (v[p] = reversed y_padded[250p : 250p+837] of the shard),
so no shift/Hankel materialization is needed on-device.  One fused DVE
tensor_tensor_reduce per output sample does multiply + sum in a single pass,
while A streams from HBM in large contiguous DMAs (memory-bound roofline).
"""

import os
import sys

import numpy as np

for _p in ("/opt/trn_rl_repo", "/opt/pypackages"):
    if _p not in sys.path:
        sys.path.append(_p)

B, T, N = 4, 64000, 588
NCORES = 8
TS = T // 2          # 32000 time steps per core shard
P = 128              # partitions
NI = TS // P         # 250 samples per partition
VW = NI + N - 1      # 837 window columns per partition
F = int(os.environ.get("K_F", "5"))       # samples per A-chunk DMA tile
BUFS = int(os.environ.get("K_BUFS", "12"))  # A-tile pool depth
NCHUNK = NI // F

_cached = {}


def _build_program():
    import concourse.bass as bass
    import concourse.tile as tile
    from concourse import bacc, mybir

    f32 = mybir.dt.float32
    nc = bacc.Bacc("TRN2", target_bir_lowering=False, debug=False,
                   num_devices=NCORES)

    a_dram = nc.dram_tensor("a_s", [TS, N], f32, kind="ExternalInput").ap()
    v_dram = nc.dram_tensor("v", [P, VW], f32, kind="ExternalInput").ap()
    r_dram = nc.dram_tensor("r", [P, NI], f32, kind="ExternalOutput").ap()

    # [TS, N] -> [P, NI, N]: partition p holds rows 250p .. 250p+249
    a_r = a_dram.rearrange("(p n) k -> p n k", p=P)

    with tile.TileContext(nc) as tc:
        with (
            tc.tile_pool(name="a", bufs=BUFS) as a_pool,
            tc.tile_pool(name="prod", bufs=2) as p_pool,
            tc.tile_pool(name="const", bufs=1) as c_pool,
        ):
            v_tile = c_pool.tile([P, VW], f32)
            # v on the ACT HWDGE ring so it doesn't delay chunk 0 on sync's
            nc.scalar.dma_start(v_tile[:], v_dram[:])
            out_tile = c_pool.tile([P, NI], f32)

            # ramp-up chunk sizes: tiny first chunks let the DVE start early
            ramp = [int(c) for c in os.environ.get("K_RAMP", "1,2,2").split(",")
                    if c.strip()]
            chunks = ramp + [F] * ((NI - sum(ramp)) // F)
            rem = NI - sum(chunks)
            if rem:
                chunks.append(rem)
            assert sum(chunks) == NI

            I = 0
            for csz in chunks:
                a_tile = a_pool.tile([P, csz * N], f32)
                nc.sync.dma_start(a_tile[:], a_r[:, I:I + csz, :])
                for i in range(csz):
                    prod = p_pool.tile([P, N], f32)
                    nc.vector.scalar_tensor_tensor(
                        out=prod[:],
                        in0=a_tile[:, i * N:(i + 1) * N],
                        scalar=1.0,
                        in1=v_tile[:, (NI - 1 - I):(NI - 1 - I) + N],
                        op0=mybir.AluOpType.mult,
                        op1=mybir.AluOpType.mult,
                        accum_out=out_tile[:, I:I + 1],
                    )
                    I += 1
            assert I == NI
            nc.sync.dma_start(r_dram[:], out_tile[:])
    nc.compile()
    return nc


def _get_program():
    if "nc" not in _cached:
        _cached["nc"] = _build_program()
    return _cached["nc"]


def _make_in_maps(y, A, zi):
    from numpy.lib.stride_tricks import sliding_window_view

    y_pad = np.concatenate([zi[:, ::-1], y], axis=1)  # [B, N+T]
    in_maps = []
    for c in range(NCORES):
        b, h = divmod(c, 2)
        base = h * TS
        a_s = A[b, base:base + TS, :]
        seg = y_pad[b, base:base + (TS - NI) + VW]          # [32587]
        v = sliding_window_view(seg, VW)[::NI]               # [128, 837]
        v = np.ascontiguousarray(v[:, ::-1])                 # reversed windows
        in_maps.append({"a_s": np.ascontiguousarray(a_s), "v": v})
    return in_maps


def _run(nc, in_maps, trace=False, **kw):
    from concourse.bass_utils import run_bass_kernel_spmd

    return run_bass_kernel_spmd(nc, in_maps, list(range(NCORES)),
                                trace=trace, **kw)


def kernel(y, A, zi):
    y = np.asarray(y, dtype=np.float32)
    A = np.asarray(A, dtype=np.float32)
    zi = np.asarray(zi, dtype=np.float32)

    nc = _get_program()
    res = _run(nc, _make_in_maps(y, A, zi))

    x = np.empty((B, T), dtype=np.float32)
    for c in range(NCORES):
        b, h = divmod(c, 2)
        base = h * TS
        r = res.results[c]["r"].reshape(TS)   # [128,250] -> t = 250p + I
        x[b, base:base + TS] = y[b, base:base + TS] + r
    return x


# revision 9
# speedup vs baseline: 1.2244x; 1.2118x over previous
"""Trainium2 Bass kernel for nn_DiffKS (time-varying FIR / InvertLPC forward).

Computes x[b,t] = y[b,t] + sum_k A[b,t,k] * y_padded[b, t+N-1-k]
with B=4, T=64000, N=588, y_padded = concat(flip(zi), y).

Sharding: 8 cores = 4 batches x 2 halves of T (each core: 32000 time steps).
Per-core layout: SBUF partition p owns 250 contiguous time steps
(t = 250*p + I, I in [0,250)).  The shifted-signal operand for output sample
I is a contiguous 588-column slice of a tiny per-partition reversed window
tile v[128, 837] (v[p] = reversed y_padded[250p : 250p+837] of the shard),
so no shift/Hankel materialization is needed on-device.  One fused DVE
tensor_tensor_reduce per output sample does multiply + sum in a single pass,
while A streams from HBM in large contiguous DMAs (memory-bound roofline).
"""

import os
import sys

import numpy as np

for _p in ("/opt/trn_rl_repo", "/opt/pypackages"):
    if _p not in sys.path:
        sys.path.append(_p)

B, T, N = 4, 64000, 588
NCORES = 8
TS = T // 2          # 32000 time steps per core shard
P = 128              # partitions
NI = TS // P         # 250 samples per partition
VW = NI + N - 1      # 837 window columns per partition
F = int(os.environ.get("K_F", "5"))       # samples per A-chunk DMA tile
BUFS = int(os.environ.get("K_BUFS", "12"))  # A-tile pool depth
NCHUNK = NI // F

_cached = {}


def _build_program():
    import concourse.bass as bass
    import concourse.tile as tile
    from concourse import bacc, mybir

    f32 = mybir.dt.float32
    nc = bacc.Bacc("TRN2", target_bir_lowering=False, debug=False,
                   num_devices=NCORES)

    a_dram = nc.dram_tensor("a_s", [TS, N], f32, kind="ExternalInput").ap()
    v_dram = nc.dram_tensor("v", [P, VW], f32, kind="ExternalInput").ap()
    r_dram = nc.dram_tensor("r", [P, NI], f32, kind="ExternalOutput").ap()

    # [TS, N] -> [P, NI, N]: partition p holds rows 250p .. 250p+249
    a_r = a_dram.rearrange("(p n) k -> p n k", p=P)

    with tile.TileContext(nc) as tc:
        with (
            tc.tile_pool(name="a", bufs=BUFS) as a_pool,
            tc.tile_pool(name="prod", bufs=2) as p_pool,
            tc.tile_pool(name="const", bufs=1) as c_pool,
        ):
            v_tile = c_pool.tile([P, VW], f32)
            v_eng = nc.scalar if os.environ.get("K_VACT", "0") == "1" else nc.sync
            v_eng.dma_start(v_tile[:], v_dram[:])
            out_tile = c_pool.tile([P, NI], f32)

            # ramp-up chunk sizes: tiny first chunks let the DVE start early
            ramp = [int(c) for c in os.environ.get("K_RAMP", "1,2,2").split(",")
                    if c.strip()]
            chunks = ramp + [F] * ((NI - sum(ramp)) // F)
            rem = NI - sum(chunks)
            if rem:
                chunks.append(rem)
            assert sum(chunks) == NI

            I = 0
            for csz in chunks:
                a_tile = a_pool.tile([P, csz * N], f32)
                nc.sync.dma_start(a_tile[:], a_r[:, I:I + csz, :])
                for i in range(csz):
                    prod = p_pool.tile([P, N], f32)
                    nc.vector.scalar_tensor_tensor(
                        out=prod[:],
                        in0=a_tile[:, i * N:(i + 1) * N],
                        scalar=1.0,
                        in1=v_tile[:, (NI - 1 - I):(NI - 1 - I) + N],
                        op0=mybir.AluOpType.mult,
                        op1=mybir.AluOpType.mult,
                        accum_out=out_tile[:, I:I + 1],
                    )
                    I += 1
            assert I == NI
            nc.sync.dma_start(r_dram[:], out_tile[:])
    nc.compile()
    return nc


def _get_program():
    if "nc" not in _cached:
        _cached["nc"] = _build_program()
    return _cached["nc"]


def _make_in_maps(y, A, zi):
    from numpy.lib.stride_tricks import sliding_window_view

    y_pad = np.concatenate([zi[:, ::-1], y], axis=1)  # [B, N+T]
    in_maps = []
    for c in range(NCORES):
        b, h = divmod(c, 2)
        base = h * TS
        a_s = A[b, base:base + TS, :]
        seg = y_pad[b, base:base + (TS - NI) + VW]          # [32587]
        v = sliding_window_view(seg, VW)[::NI]               # [128, 837]
        v = np.ascontiguousarray(v[:, ::-1])                 # reversed windows
        in_maps.append({"a_s": np.ascontiguousarray(a_s), "v": v})
    return in_maps


def _run(nc, in_maps, trace=False, **kw):
    from concourse.bass_utils import run_bass_kernel_spmd

    return run_bass_kernel_spmd(nc, in_maps, list(range(NCORES)),
                                trace=trace, **kw)


def kernel(y, A, zi):
    y = np.asarray(y, dtype=np.float32)
    A = np.asarray(A, dtype=np.float32)
    zi = np.asarray(zi, dtype=np.float32)

    nc = _get_program()
    res = _run(nc, _make_in_maps(y, A, zi))

    x = np.empty((B, T), dtype=np.float32)
    for c in range(NCORES):
        b, h = divmod(c, 2)
        base = h * TS
        r = res.results[c]["r"].reshape(TS)   # [128,250] -> t = 250p + I
        x[b, base:base + TS] = y[b, base:base + TS] + r
    return x
